# revision 28
# baseline (speedup 1.0000x reference)
"""MoE top-k routing + grouped down-proj GEMM + reduce-scatter for trn2 (8 cores).

Problem: intermediate_states [4, 2048, 1024] f16 (rank-sharded expanded-token
activations), w [4, 8, 1024, 2048] f16 (rank-sharded per-expert down-proj),
router_logits [1024, 8] f32, topk=2.  Output [4, 256, 2048] f16.

Strategy (W-stationary): for each expanded token tk routed to expert e(tk),
y_part[tk] = (gate(tk) * x_full[tk]) @ w_full[e(tk)] with x_full [TK, 4096]
(rank dim folded into the contraction) and w_full[e] [4096, 2048].  The gate
is folded into x on the host (f16 rounding ~5e-4, far under tolerance).

Unlike an x-stationary decomposition (cost quantized at whole 128-token
tiles x 2048 H-columns), the W panel [128k, 128h] is loaded as the matmul
STATIONARY operand and the tokens stream as the MOVING operand, so PE cost is
proportional to the actual per-expert token count.  LDWEIGHTS overlaps the
matmuls via the PE's 64-deep reorder window + FWL, and the cost model charges
moving columns only.

Work layout: the 16 (expert, K-half) groups are placed on 8 cores x 2 slots.
Slot A takes the 2 K-halves of each of the 4 biggest experts, slot B the 4
smallest; slot stream capacities CA/CB = exact max assigned count (program
is SPMD-identical across cores, so capacities are per-slot-class; for the
graded routing CA+CB = 545 vs the ideal 512 -- ~6% padding, vs ~25% for
128-token x-stationary tiles).  Per (slot, h-tile of 128 H cols): accumulate
16 K-subtile panels into one PSUM bank [128h, cap<=512 tokens] f32, evict
f32->f16 (scalar/vector engines alternating), DMA out every 2 h-tiles.
Host sums each expanded token's 2 K-half partials and the topk pair.

DMA: all DRAM parameters are laid out partition-major so every transfer is a
contiguous [128, X] copy (one long run per partition).  W streams as
256KB-1MB chunks on the sync (SP HWDGE) ring in consumption order (HWDGE
drains FIFO per ring, so emission order = arrival order); outputs go on the
scalar (ACT HWDGE) ring.  ~37 DMAs total (vs 53 in the x-stationary version
whose serialized ~0.65us issues throttled the stream).  Per-core traffic is
16MB W + 2.2MB x + 2.2MB out vs ~358GB/s fair-share HBM -- the kernel sits
essentially at both the PE roofline (58us of moving columns) and the HBM
roofline (57us).

A short warmup matmul chain on a zeroed tile covers the HAM clock-ramp window
while the first W/x chunks land.

The program depends on (CA, CB) only and is cached on that key, so the
deterministic graded input compiles exactly once; any routing is handled
correctly (capacities adapt, column streams >512 are chunked per PSUM bank).
"""

import numpy as np

R, T_TOK, E = 4, 1024, 8
I_PR, H = 1024, 2048
K = R * I_PR            # 4096 contraction
KH = K // 2             # 2048 per K-half
P = 128
NKS = KH // P           # 16 k-subtiles per K-half slot
NHT = H // P            # 16 h-tiles
N_CORES = 8
NWARM = 12              # HAM warmup matmuls (N=512, cold ~427ns each):
                        # ~5.1us of guaranteed PE busy covers the HAM
                        # un-throttle window (~3.4us sustained) and most of
                        # the DMA stream head (xa + W h0/h1, ~2.2MB), so
                        # real matmuls start warm with few trickle-stalls.

_prog_cache: dict[tuple, object] = {}


def _new_bacc():
    from concourse import bacc

    return bacc.Bacc(
        "TRN2",
        target_bir_lowering=False,
        debug=False,
        num_devices=N_CORES,
    )


def _col_chunks(cap):
    """Split a token stream into <=512-column chunks (one PSUM bank each)."""
    return [(o, min(512, cap - o)) for o in range(0, cap, 512)]


def _build_program(ca, cb):
    import concourse.mybir as mybir
    import concourse.tile as tile

    f16 = mybir.dt.float16
    f32 = mybir.dt.float32

    nc = _new_bacc()
    # Per-partition layouts (partition dim first, everything contiguous):
    #   w*: [p=128, (h=16, ks=16, hh=128)]   w[p, h, ks, hh] = W[ks*128+p, h*128+hh]
    #   x*: [p=128, (ks=16, t=cap)]          x[p, ks, t] = xg[tok_t, ks*128+p]
    #   ho*:[p=128, (h=16, t=cap)]           ho[p, h, t] = y_part[tok_t, h*128+p]
    wa = nc.declare_dram_parameter("wa", [P, NHT * NKS * P], f16, isOutput=False)
    wb = nc.declare_dram_parameter("wb", [P, NHT * NKS * P], f16, isOutput=False)
    xa = nc.declare_dram_parameter("xa", [P, NKS * ca], f16, isOutput=False)
    xb = nc.declare_dram_parameter("xb", [P, NKS * cb], f16, isOutput=False)
    hoa = nc.declare_dram_parameter("hoa", [P, NHT * ca], f16, isOutput=True)
    hob = nc.declare_dram_parameter("hob", [P, NHT * cb], f16, isOutput=True)

    HB = NKS * P            # 2048: one h-tile worth of W per partition

    with tile.TileContext(nc) as tc:
        with tc.tile_pool(name="sb", bufs=1) as sb, \
             tc.tile_pool(name="ps", bufs=2, space="PSUM") as psp:
            # W tiles: one per (slot, hc) covering 2 h-tiles = 512KB each.
            wat = [sb.tile([P, 2 * HB], f16, name=f"wa{c}", tag=f"wa{c}",
                           bufs=1) for c in range(8)]
            wbt = [sb.tile([P, 2 * HB], f16, name=f"wb{c}", tag=f"wb{c}",
                           bufs=1) for c in range(8)]
            xat = sb.tile([P, NKS * ca], f16, name="xa", tag="xa", bufs=1)
            xbt = sb.tile([P, NKS * cb], f16, name="xb", tag="xb", bufs=1)
            warm = sb.tile([P, 512], f16, name="warm", tag="warm", bufs=1)

            def dma_w(slot, hc, elo, ehi):
                # elo/ehi: element range within the tile's 2*HB free dim
                src = wa if slot == 0 else wb
                t = (wat if slot == 0 else wbt)[hc]
                lo = hc * 2 * HB
                nc.sync.dma_start(t[:, elo:ehi], src[:, lo + elo:lo + ehi])

            def dma_x(slot, kslo, ksn):
                src, t, cap = (xa, xat, ca) if slot == 0 else (xb, xbt, cb)
                nc.sync.dma_start(t[:, kslo * cap:(kslo + ksn) * cap],
                                  src[:, kslo * cap:(kslo + ksn) * cap])

            # Issue order = consumption order on the SP ring (drains FIFO).
            # The first h-pass can't finish before xa + W[h0] have all
            # arrived, so xa chunks lead the stream (the first h-pass runs
            # at the cold PE clock, which roughly matches the arrival rate);
            # steady state is 1MB of W per 2 h-passes, arriving ~25% faster
            # than the PE consumes it.  xb rides mid-stream -- slot B compute
            # starts after all 16 A passes.
            dma_x(0, 0, 4)                        # xa ks0-3
            dma_w(0, 0, 0, HB // 2)               # W A h0, ks0-7 (256KB)
            dma_x(0, 4, 4)                        # xa ks4-7
            dma_w(0, 0, HB // 2, HB)              # W A h0, ks8-15
            dma_x(0, 8, 4)                        # xa ks8-11
            dma_x(0, 12, 4)                       # xa ks12-15
            dma_w(0, 0, HB, 2 * HB)               # W A h1 (512KB)
            for hc in range(1, 8):
                dma_w(0, hc, 0, 2 * HB)           # W A 1MB chunks
            dma_x(1, 0, NKS)                      # xb (B starts ~35us later)
            for hc in range(8):
                dma_w(1, hc, 0, 2 * HB)           # W B 1MB chunks

            # HAM warmup: keep the PE busy (and ramping to 2.4GHz) while the
            # first DMAs land.  Garbage goes to a dedicated PSUM bank.
            nc.vector.memset(warm[:], 0.0)
            wps = psp.tile([P, 512], f32, name="wps", tag="wps", bufs=1)
            for i in range(NWARM):
                nc.tensor.matmul(wps[:], lhsT=warm[:, :P], rhs=warm[:],
                                 start=(i == 0), stop=(i == NWARM - 1))

            for slot in range(2):
                cap = ca if slot == 0 else cb
                wt = wat if slot == 0 else wbt
                xt = xat if slot == 0 else xbt
                ho = hoa if slot == 0 else hob
                ots = {}
                for h in range(NHT):
                    hc, hi = divmod(h, 2)
                    if hi == 0:
                        ots[hc] = sb.tile([P, 2 * cap], f16,
                                          name=f"o{slot}_{hc}",
                                          tag=f"o{slot}", bufs=3)
                    ot = ots[hc]
                    for (co, cn) in _col_chunks(cap):
                        ps = psp.tile([P, 512], f32, name=f"ps{slot}_{h}_{co}",
                                      tag="ps", bufs=6)
                        for ks in range(NKS):
                            nc.tensor.matmul(
                                ps[:, :cn],
                                lhsT=wt[hc][:, (hi * NKS + ks) * P:
                                            (hi * NKS + ks + 1) * P],
                                rhs=xt[:, ks * cap + co:ks * cap + co + cn],
                                start=(ks == 0),
                                stop=(ks == NKS - 1),
                            )
                        dst = ot[:, hi * cap + co:hi * cap + co + cn]
                        # alternate eviction engines so they pipeline; the
                        # very last pass evicts on scalar so the final out
                        # DMA (also issued by scalar) needs no cross-engine
                        # semaphore hop.
                        last_pass = slot == 1 and h == NHT - 1
                        if h % 2 and not last_pass:
                            nc.vector.tensor_copy(dst, ps[:, :cn])
                        else:
                            nc.scalar.activation(
                                dst, ps[:, :cn],
                                mybir.ActivationFunctionType.Copy)
                    if slot == 1 and hc == NHT // 2 - 1:
                        # tail: per-h out DMAs so the last transfer is tiny
                        nc.scalar.dma_start(
                            ho[:, h * cap:(h + 1) * cap],
                            ot[:, hi * cap:(hi + 1) * cap])
                    elif hi == 1:
                        nc.scalar.dma_start(
                            ho[:, hc * 2 * cap:(hc + 1) * 2 * cap], ot[:])
    nc.finalize()
    return nc


def _get_program(ca, cb):
    key = (ca, cb)
    if key not in _prog_cache:
        _prog_cache[key] = _build_program(ca, cb)
    return _prog_cache[key]


def _route(logits, topk):
    """numpy replica of jax.lax.top_k + softmax over selected logits."""
    idx = np.argsort(-logits, axis=-1, kind="stable")[:, :topk]      # [T, topk]
    vals = np.take_along_axis(logits, idx, axis=-1)
    mx = vals.max(-1, keepdims=True)
    gate = np.exp(vals - mx)
    gate = gate / gate.sum(-1, keepdims=True)                        # f32
    return idx, gate


def prepare(inputs):
    """Host routing + per-core input construction.

    Returns (nc, launches, combine): launches is a list of per-launch in_maps
    (one dict per core); combine(list_of_per_launch_results) -> final output.
    """
    x = np.asarray(inputs["intermediate_states"])          # [R, TK, I_PR] f16
    w = np.asarray(inputs["w"])                            # [R, E, I_PR, H] f16
    logits = np.asarray(inputs["router_logits"]).astype(np.float32)  # [T, E]
    topk = int(np.asarray(inputs["topk"]))

    T, E_ = logits.shape
    TK = T * topk
    assert x.shape == (R, TK, I_PR) and w.shape == (R, E_, I_PR, H) and E_ == E

    idx, gate = _route(logits, topk)
    flat_e = idx.reshape(-1)                               # expert of tk
    counts = np.bincount(flat_e, minlength=E)
    starts = np.zeros(E + 1, np.int64)
    starts[1:] = np.cumsum(counts)
    order = np.argsort(flat_e, kind="stable")              # tks sorted by expert
    g_flat = gate.reshape(TK).astype(np.float32)

    # gate folded into x rows: [TK, 4096] f16
    xf = np.ascontiguousarray(x.transpose(1, 0, 2)).reshape(TK, K)
    xg = (xf.astype(np.float32) * g_flat[:, None]).astype(np.float16)

    # slot assignment: the 4 biggest experts' K-halves fill the A slots of
    # the 8 cores, the 4 smallest fill the B slots.
    rank = np.argsort(-counts, kind="stable")
    bigs, smalls = rank[:4], rank[4:]

    ca = max(16, int(counts[bigs].max()))
    cb = max(16, int(counts[smalls].max()))
    nc = _get_program(ca, cb)

    slotA = [(int(bigs[i // 2]), i % 2) for i in range(N_CORES)]
    slotB = [(int(smalls[i // 2]), i % 2) for i in range(N_CORES)]

    in_maps = []
    meta = []
    for c in range(N_CORES):
        m = {}
        mm = {}
        for (e, kh), cap, xnm, wnm, snm in (
                (slotA[c], ca, "xa", "wa", "a"),
                (slotB[c], cb, "xb", "wb", "b")):
            toks = order[starts[e]:starts[e + 1]]
            n = len(toks)
            xarr = np.zeros((P, NKS, cap), np.float16)
            if n:
                xs = xg[toks, kh * KH:(kh + 1) * KH]       # [n, 2048]
                xarr[:, :, :n] = xs.reshape(n, NKS, P).transpose(2, 1, 0)
            m[xnm] = np.ascontiguousarray(xarr.reshape(P, NKS * cap))
            wsl = w[2 * kh:2 * kh + 2, e].reshape(KH, H)   # [2048, 2048] f16
            wv = wsl.reshape(NKS, P, NHT, P).transpose(1, 2, 0, 3)
            m[wnm] = np.ascontiguousarray(wv).reshape(P, NHT * NKS * P)
            mm[snm] = (toks, n, cap)
        in_maps.append(m)
        meta.append(mm)

    launches = [in_maps]

    def combine(all_results):
        res = all_results[0]
        y = np.zeros((TK, H), np.float32)
        for c in range(N_CORES):
            for snm, honm in (("a", "hoa"), ("b", "hob")):
                toks, n, cap = meta[c][snm]
                if not n:
                    continue
                hoarr = res[c][honm].reshape(P, NHT, cap)[:, :, :n]
                y[toks] += hoarr.transpose(2, 1, 0).reshape(n, H)
        yt = y.reshape(T, topk, H).sum(axis=1)
        return yt.astype(np.float16).reshape(R, T // R, H)

    return nc, launches, combine


def kernel(**inputs) -> np.ndarray:
    nc, launches, combine = prepare(inputs)
    from concourse.bass_utils import run_bass_kernel_spmd

    all_results = []
    for in_maps in launches:
        res = run_bass_kernel_spmd(nc, in_maps, core_ids=list(range(N_CORES)))
        all_results.append(res.results)
    return combine(all_results)


# revision 29
# speedup vs baseline: 1.0388x; 1.0388x over previous
"""MoE top-k routing + grouped down-proj GEMM + reduce-scatter for trn2 (8 cores).

Problem: intermediate_states [4, 2048, 1024] f16 (rank-sharded expanded-token
activations), w [4, 8, 1024, 2048] f16 (rank-sharded per-expert down-proj),
router_logits [1024, 8] f32, topk=2.  Output [4, 256, 2048] f16.

Strategy (W-stationary): for each expanded token tk routed to expert e(tk),
y_part[tk] = (gate(tk) * x_full[tk]) @ w_full[e(tk)] with x_full [TK, 4096]
(rank dim folded into the contraction) and w_full[e] [4096, 2048].  The gate
is folded into x on the host (f16 rounding ~5e-4, far under tolerance).

Unlike an x-stationary decomposition (cost quantized at whole 128-token
tiles x 2048 H-columns), the W panel [128k, 128h] is loaded as the matmul
STATIONARY operand and the tokens stream as the MOVING operand, so PE cost is
proportional to the actual per-expert token count.  LDWEIGHTS overlaps the
matmuls via the PE's 64-deep reorder window + FWL, and the cost model charges
moving columns only.

Work layout: the 16 (expert, K-half) groups are placed on 8 cores x 2 slots.
Slot A takes the 2 K-halves of each of the 4 biggest experts, slot B the 4
smallest; slot stream capacities CA/CB = exact max assigned count (program
is SPMD-identical across cores, so capacities are per-slot-class; for the
graded routing CA+CB = 545 vs the ideal 512 -- ~6% padding, vs ~25% for
128-token x-stationary tiles).  Per (slot, h-tile of 128 H cols): accumulate
16 K-subtile panels into one PSUM bank [128h, cap<=512 tokens] f32, evict
f32->f16 (scalar/vector engines alternating), DMA out every 2 h-tiles.
Host sums each expanded token's 2 K-half partials and the topk pair.

DMA: all DRAM parameters are laid out partition-major so every transfer is a
contiguous [128, X] copy (one long run per partition).  W streams as
256KB-1MB chunks on the sync (SP HWDGE) ring in consumption order (HWDGE
drains FIFO per ring, so emission order = arrival order); outputs go on the
scalar (ACT HWDGE) ring.  ~37 DMAs total (vs 53 in the x-stationary version
whose serialized ~0.65us issues throttled the stream).  Per-core traffic is
16MB W + 2.2MB x + 2.2MB out vs ~358GB/s fair-share HBM -- the kernel sits
essentially at both the PE roofline (58us of moving columns) and the HBM
roofline (57us).

A short warmup matmul chain on a zeroed tile covers the HAM clock-ramp window
while the first W/x chunks land.

The program depends on (CA, CB) only and is cached on that key, so the
deterministic graded input compiles exactly once; any routing is handled
correctly (capacities adapt, column streams >512 are chunked per PSUM bank).
"""

import numpy as np

R, T_TOK, E = 4, 1024, 8
I_PR, H = 1024, 2048
K = R * I_PR            # 4096 contraction
KH = K // 2             # 2048 per K-half
P = 128
NKS = KH // P           # 16 k-subtiles per K-half slot
NHT = H // P            # 16 h-tiles
N_CORES = 8
NWARM = 16              # HAM warmup matmuls (N=512, cold ~427ns each):
                        # ~6.8us of guaranteed PE busy covers the HAM
                        # un-throttle window (~3.4us sustained) and most of
                        # the DMA stream head (xa + W h0/h1, ~2.2MB), so
                        # real matmuls start warm with few trickle-stalls.

_prog_cache: dict[tuple, object] = {}


def _new_bacc():
    from concourse import bacc

    return bacc.Bacc(
        "TRN2",
        target_bir_lowering=False,
        debug=False,
        num_devices=N_CORES,
    )


def _col_chunks(cap):
    """Split a token stream into <=512-column chunks (one PSUM bank each)."""
    return [(o, min(512, cap - o)) for o in range(0, cap, 512)]


def _build_program(ca, cb):
    import concourse.mybir as mybir
    import concourse.tile as tile

    f16 = mybir.dt.float16
    f32 = mybir.dt.float32

    nc = _new_bacc()
    # Per-partition layouts (partition dim first, everything contiguous):
    #   w*: [p=128, (h=16, ks=16, hh=128)]   w[p, h, ks, hh] = W[ks*128+p, h*128+hh]
    #   x*: [p=128, (ks=16, t=cap)]          x[p, ks, t] = xg[tok_t, ks*128+p]
    #   ho*:[p=128, (h=16, t=cap)]           ho[p, h, t] = y_part[tok_t, h*128+p]
    wa = nc.declare_dram_parameter("wa", [P, NHT * NKS * P], f16, isOutput=False)
    wb = nc.declare_dram_parameter("wb", [P, NHT * NKS * P], f16, isOutput=False)
    xa = nc.declare_dram_parameter("xa", [P, NKS * ca], f16, isOutput=False)
    xb = nc.declare_dram_parameter("xb", [P, NKS * cb], f16, isOutput=False)
    hoa = nc.declare_dram_parameter("hoa", [P, NHT * ca], f16, isOutput=True)
    hob = nc.declare_dram_parameter("hob", [P, NHT * cb], f16, isOutput=True)

    HB = NKS * P            # 2048: one h-tile worth of W per partition

    with tile.TileContext(nc) as tc:
        with tc.tile_pool(name="sb", bufs=1) as sb, \
             tc.tile_pool(name="ps", bufs=2, space="PSUM") as psp:
            # W tiles: one per (slot, hc) covering 2 h-tiles = 512KB each.
            wat = [sb.tile([P, 2 * HB], f16, name=f"wa{c}", tag=f"wa{c}",
                           bufs=1) for c in range(8)]
            wbt = [sb.tile([P, 2 * HB], f16, name=f"wb{c}", tag=f"wb{c}",
                           bufs=1) for c in range(8)]
            xat = sb.tile([P, NKS * ca], f16, name="xa", tag="xa", bufs=1)
            xbt = sb.tile([P, NKS * cb], f16, name="xb", tag="xb", bufs=1)
            warm = sb.tile([P, 512], f16, name="warm", tag="warm", bufs=1)

            def dma_w(slot, hc, elo, ehi):
                # elo/ehi: element range within the tile's 2*HB free dim
                src = wa if slot == 0 else wb
                t = (wat if slot == 0 else wbt)[hc]
                lo = hc * 2 * HB
                nc.sync.dma_start(t[:, elo:ehi], src[:, lo + elo:lo + ehi])

            def dma_x(slot, kslo, ksn):
                src, t, cap = (xa, xat, ca) if slot == 0 else (xb, xbt, cb)
                nc.sync.dma_start(t[:, kslo * cap:(kslo + ksn) * cap],
                                  src[:, kslo * cap:(kslo + ksn) * cap])

            # Issue order = consumption order on the SP ring (drains FIFO).
            # The first h-pass can't finish before xa + W[h0] have all
            # arrived, so xa chunks lead the stream (the first h-pass runs
            # at the cold PE clock, which roughly matches the arrival rate);
            # steady state is 1MB of W per 2 h-passes, arriving ~25% faster
            # than the PE consumes it.  xb rides mid-stream -- slot B compute
            # starts after all 16 A passes.
            dma_x(0, 0, 4)                        # xa ks0-3
            dma_w(0, 0, 0, HB // 2)               # W A h0, ks0-7 (256KB)
            dma_x(0, 4, 4)                        # xa ks4-7
            dma_w(0, 0, HB // 2, HB)              # W A h0, ks8-15
            dma_x(0, 8, 4)                        # xa ks8-11
            dma_x(0, 12, 4)                       # xa ks12-15
            dma_w(0, 0, HB, 2 * HB)               # W A h1 (512KB)
            for hc in range(1, 8):
                dma_w(0, hc, 0, 2 * HB)           # W A 1MB chunks
            dma_x(1, 0, NKS)                      # xb (B starts ~35us later)
            for hc in range(8):
                dma_w(1, hc, 0, 2 * HB)           # W B 1MB chunks

            # HAM warmup: keep the PE busy (and ramping to 2.4GHz) while the
            # first DMAs land.  Garbage goes to a dedicated PSUM bank.
            nc.vector.memset(warm[:], 0.0)
            wps = psp.tile([P, 512], f32, name="wps", tag="wps", bufs=1)
            for i in range(NWARM):
                nc.tensor.matmul(wps[:], lhsT=warm[:, :P], rhs=warm[:],
                                 start=(i == 0), stop=(i == NWARM - 1))

            for slot in range(2):
                cap = ca if slot == 0 else cb
                wt = wat if slot == 0 else wbt
                xt = xat if slot == 0 else xbt
                ho = hoa if slot == 0 else hob
                ots = {}
                for h in range(NHT):
                    hc, hi = divmod(h, 2)
                    if hi == 0:
                        ots[hc] = sb.tile([P, 2 * cap], f16,
                                          name=f"o{slot}_{hc}",
                                          tag=f"o{slot}", bufs=4)
                    ot = ots[hc]
                    for (co, cn) in _col_chunks(cap):
                        ps = psp.tile([P, 512], f32, name=f"ps{slot}_{h}_{co}",
                                      tag="ps", bufs=7)
                        for ks in range(NKS):
                            nc.tensor.matmul(
                                ps[:, :cn],
                                lhsT=wt[hc][:, (hi * NKS + ks) * P:
                                            (hi * NKS + ks + 1) * P],
                                rhs=xt[:, ks * cap + co:ks * cap + co + cn],
                                start=(ks == 0),
                                stop=(ks == NKS - 1),
                            )
                        dst = ot[:, hi * cap + co:hi * cap + co + cn]
                        # alternate eviction engines so they pipeline; the
                        # very last pass evicts on scalar so the final out
                        # DMA (also issued by scalar) needs no cross-engine
                        # semaphore hop.
                        last_pass = slot == 1 and h == NHT - 1
                        if h % 2 and not last_pass:
                            nc.vector.tensor_copy(dst, ps[:, :cn])
                        else:
                            nc.scalar.activation(
                                dst, ps[:, :cn],
                                mybir.ActivationFunctionType.Copy)
                    if slot == 1 and hc == NHT // 2 - 1:
                        # tail: per-h out DMAs so the last transfer is tiny
                        nc.scalar.dma_start(
                            ho[:, h * cap:(h + 1) * cap],
                            ot[:, hi * cap:(hi + 1) * cap])
                    elif hi == 1:
                        nc.scalar.dma_start(
                            ho[:, hc * 2 * cap:(hc + 1) * 2 * cap], ot[:])
    nc.finalize()
    return nc


def _get_program(ca, cb):
    key = (ca, cb)
    if key not in _prog_cache:
        _prog_cache[key] = _build_program(ca, cb)
    return _prog_cache[key]


def _route(logits, topk):
    """numpy replica of jax.lax.top_k + softmax over selected logits."""
    idx = np.argsort(-logits, axis=-1, kind="stable")[:, :topk]      # [T, topk]
    vals = np.take_along_axis(logits, idx, axis=-1)
    mx = vals.max(-1, keepdims=True)
    gate = np.exp(vals - mx)
    gate = gate / gate.sum(-1, keepdims=True)                        # f32
    return idx, gate


def prepare(inputs):
    """Host routing + per-core input construction.

    Returns (nc, launches, combine): launches is a list of per-launch in_maps
    (one dict per core); combine(list_of_per_launch_results) -> final output.
    """
    x = np.asarray(inputs["intermediate_states"])          # [R, TK, I_PR] f16
    w = np.asarray(inputs["w"])                            # [R, E, I_PR, H] f16
    logits = np.asarray(inputs["router_logits"]).astype(np.float32)  # [T, E]
    topk = int(np.asarray(inputs["topk"]))

    T, E_ = logits.shape
    TK = T * topk
    assert x.shape == (R, TK, I_PR) and w.shape == (R, E_, I_PR, H) and E_ == E

    idx, gate = _route(logits, topk)
    flat_e = idx.reshape(-1)                               # expert of tk
    counts = np.bincount(flat_e, minlength=E)
    starts = np.zeros(E + 1, np.int64)
    starts[1:] = np.cumsum(counts)
    order = np.argsort(flat_e, kind="stable")              # tks sorted by expert
    g_flat = gate.reshape(TK).astype(np.float32)

    # gate folded into x rows: [TK, 4096] f16
    xf = np.ascontiguousarray(x.transpose(1, 0, 2)).reshape(TK, K)
    xg = (xf.astype(np.float32) * g_flat[:, None]).astype(np.float16)

    # slot assignment: the 4 biggest experts' K-halves fill the A slots of
    # the 8 cores, the 4 smallest fill the B slots.
    rank = np.argsort(-counts, kind="stable")
    bigs, smalls = rank[:4], rank[4:]

    ca = max(16, int(counts[bigs].max()))
    cb = max(16, int(counts[smalls].max()))
    nc = _get_program(ca, cb)

    slotA = [(int(bigs[i // 2]), i % 2) for i in range(N_CORES)]
    slotB = [(int(smalls[i // 2]), i % 2) for i in range(N_CORES)]

    in_maps = []
    meta = []
    for c in range(N_CORES):
        m = {}
        mm = {}
        for (e, kh), cap, xnm, wnm, snm in (
                (slotA[c], ca, "xa", "wa", "a"),
                (slotB[c], cb, "xb", "wb", "b")):
            toks = order[starts[e]:starts[e + 1]]
            n = len(toks)
            xarr = np.zeros((P, NKS, cap), np.float16)
            if n:
                xs = xg[toks, kh * KH:(kh + 1) * KH]       # [n, 2048]
                xarr[:, :, :n] = xs.reshape(n, NKS, P).transpose(2, 1, 0)
            m[xnm] = np.ascontiguousarray(xarr.reshape(P, NKS * cap))
            wsl = w[2 * kh:2 * kh + 2, e].reshape(KH, H)   # [2048, 2048] f16
            wv = wsl.reshape(NKS, P, NHT, P).transpose(1, 2, 0, 3)
            m[wnm] = np.ascontiguousarray(wv).reshape(P, NHT * NKS * P)
            mm[snm] = (toks, n, cap)
        in_maps.append(m)
        meta.append(mm)

    launches = [in_maps]

    def combine(all_results):
        res = all_results[0]
        y = np.zeros((TK, H), np.float32)
        for c in range(N_CORES):
            for snm, honm in (("a", "hoa"), ("b", "hob")):
                toks, n, cap = meta[c][snm]
                if not n:
                    continue
                hoarr = res[c][honm].reshape(P, NHT, cap)[:, :, :n]
                y[toks] += hoarr.transpose(2, 1, 0).reshape(n, H)
        yt = y.reshape(T, topk, H).sum(axis=1)
        return yt.astype(np.float16).reshape(R, T // R, H)

    return nc, launches, combine


def kernel(**inputs) -> np.ndarray:
    nc, launches, combine = prepare(inputs)
    from concourse.bass_utils import run_bass_kernel_spmd

    all_results = []
    for in_maps in launches:
        res = run_bass_kernel_spmd(nc, in_maps, core_ids=list(range(N_CORES)))
        all_results.append(res.results)
    return combine(all_results)
